# revision 23
# baseline (speedup 1.0000x reference)
"""Masked-fill kernel for Trainium2 (8 NeuronCores, data-parallel over batch).

reference semantics:
    masked_x = x.copy(); masked_x[b, mask_indices[b], :] = emb_mask[0]
    return masked_x, mask_indices

Strategy: shard x along batch (4 rows per core). On the host, expand
mask_indices into a dense {0,1} uint32 mask over the N axis (cheap: 256K
entries; idempotent under duplicate indices). On-device, each core streams
its 32 MiB shard through SBUF in 1 MiB tiles (loads on the sync HWDGE
ring, stores on the scalar HWDGE ring, 20 tile buffers) and applies one
DVE copy_predicated per tile (mask broadcast over DIM, emb broadcast over
the row-chunk axis) between load and store. Total HBM traffic per core =
32 MiB read + 32 MiB write + ~0.25 MiB of mask/emb — the memory roofline
for this op. Measured ~200 us/exec on silicon (~390 GB/s/core combined
read+write), vs ~187 us theoretical at the 358 GB/s HBM-per-core limit.
"""

import numpy as np

B, N, DIM = 32, 8192, 256
N_CORES = 8
RPC = B // N_CORES  # batch rows per core = 4
P = 128             # SBUF partitions
CHUNKS = 8          # tiles per batch row
NH = N // CHUNKS    # 4096 rows of x per tile
T = NH // P         # 32 rows per partition per tile
MASK_COLS = RPC * CHUNKS * T  # 256

_cached = {}


def _chunk_plan(chunks, taper):
    """Per-row list of chunk sizes in t-units (t = rows-per-partition).
    Row plan sums to N//P (=64). With taper, the first row starts with
    small chunks (fast pipeline fill) and the last row ends with small
    chunks (fast drain); middle rows stay uniform."""
    t = N // P // chunks
    base = [t] * chunks
    if not taper:
        return [list(base) for _ in range(RPC)]
    assert chunks == 8, "taper plan is tuned for chunks=8"
    head = [2, 2, 4] + [8] * 7          # sums to 64
    tail = [8] * 7 + [4, 2, 2]          # sums to 64
    plans = [list(base) for _ in range(RPC)]
    plans[0] = head
    plans[RPC - 1] = tail
    return plans


def _build_nc(chunks=CHUNKS, bufs=20, load_eng="sync", store_eng="scalar",
              predicate=True, bench_loop=0, const_eng="sync", alternate=False,
              third="none", order="rh", taper=False, unroll=1):
    """Build the bass module.

    bench_loop=0: the real kernel (x -> out, one pass).
    bench_loop=K: benchmark variant — a hardware For_i loop runs the body K
    times (x -> scratch), then one small DMA keeps scratch live by writing
    the tiny external output. Used only for timing.
    """
    import concourse.bacc as bacc
    import concourse.mybir as mybir
    import concourse.tile as tile

    nh = N // chunks
    t = nh // P

    nc = bacc.Bacc("TRN2", target_bir_lowering=False, debug=False)
    f32 = mybir.dt.float32
    u32 = mybir.dt.uint32

    x_in = nc.dram_tensor("x", [RPC, N, DIM], f32, kind="ExternalInput")
    mask_in = nc.dram_tensor("mask", [P, MASK_COLS], u32, kind="ExternalInput")
    emb_in = nc.dram_tensor("emb", [P, DIM], f32, kind="ExternalInput")
    if bench_loop:
        out = nc.dram_tensor("out", [P, DIM], f32, kind="ExternalOutput")
        scratch = nc.dram_tensor("scratch", [RPC, N, DIM], f32)
    else:
        out = nc.dram_tensor("out", [RPC, N, DIM], f32, kind="ExternalOutput")
        scratch = None

    with tile.TileContext(nc) as tc:
        with (
            tc.tile_pool(name="const", bufs=1) as cpool,
            tc.tile_pool(name="big", bufs=bufs) as pool,
        ):
            mask_t = cpool.tile([P, MASK_COLS], u32)
            emb_t = cpool.tile([P, DIM], f32)
            cload = getattr(nc, const_eng)
            cload.dma_start(out=mask_t[:], in_=mask_in[:])
            cload.dma_start(out=emb_t[:], in_=emb_in[:])
            load = getattr(nc, load_eng)
            store = getattr(nc, store_eng)

            plans = _chunk_plan(chunks, taper)
            # work item: (row, x-row start, chunk size t_i, mask col start)
            items = []
            for r in range(RPC):
                cum = 0
                for t_i in plans[r]:
                    items.append((r, P * cum, t_i, r * (N // P) + cum))
                    cum += t_i
            if order == "hr":
                assert not taper
                items = [
                    items[r * chunks + h]
                    for h in range(chunks)
                    for r in range(RPC)
                ]

            def body(tgt):
                for i, (r, s, t_i, c0) in enumerate(items):
                    ld = (getattr(nc, store_eng) if (alternate and i % 2)
                          else load)
                    st = (getattr(nc, load_eng) if (alternate and i % 2)
                          else store)
                    if third == "load" and i % 3 == 2:
                        ld = nc.gpsimd
                    elif third == "store" and i % 3 == 2:
                        st = nc.gpsimd
                    nrow = P * t_i
                    buf = pool.tile([P, t * DIM], f32, tag="xbuf")
                    dst3 = buf[:, :t_i * DIM].rearrange(
                        "p (t d) -> p t d", d=DIM
                    )
                    src = x_in[r, s:s + nrow, :].rearrange(
                        "(p t) d -> p t d", p=P
                    )
                    ld.dma_start(out=dst3, in_=src)

                    # mask for x rows [s, s+P*t_i): dense mask for batch
                    # row r occupies cols [r*(N//P), (r+1)*(N//P)); this
                    # chunk starts at col offset cum (= s // P)
                    if predicate:
                        nc.vector.copy_predicated(
                            out=dst3,
                            mask=mask_t[:, c0:c0 + t_i].unsqueeze(2)
                            .broadcast_to([P, t_i, DIM]),
                            data=emb_t[:].unsqueeze(1).broadcast_to(
                                [P, t_i, DIM]
                            ),
                        )

                    dsto = tgt[r, s:s + nrow, :].rearrange(
                        "(p t) d -> p t d", p=P
                    )
                    st.dma_start(out=dsto, in_=dst3)

            if bench_loop:
                with tc.For_i(0, bench_loop, 1):
                    for _ in range(unroll):
                        body(scratch)
                # keep scratch live: copy one row-chunk out through SBUF
                sm = cpool.tile([P, DIM], f32)
                nc.sync.dma_start(
                    out=sm[:],
                    in_=scratch[0, 0:P, :].rearrange("(p o) d -> p (o d)", p=P),
                )
                nc.sync.dma_start(out=out[:], in_=sm[:])
            else:
                body(out)
    nc.compile()
    return nc


def _get_nc():
    if "nc" not in _cached:
        _cached["nc"] = _build_nc()
    return _cached["nc"]


def _host_prep(x, mask_indices, emb_mask, chunks=CHUNKS, taper=False):
    """Build per-core input maps. The mask layout must mirror the device
    chunk plan: for each chunk of t_i rows-per-partition starting at x row
    s, partition p covers rows s + p*t_i + tt, so the chunk's mask block
    is dense[r, s:s+P*t_i].reshape(P, t_i), concatenated along columns."""
    plans = _chunk_plan(chunks, taper)
    dense = np.zeros((B, N), dtype=np.uint32)
    dense[np.arange(B)[:, None], mask_indices.astype(np.int64)] = 1
    emb_b = np.ascontiguousarray(
        np.broadcast_to(emb_mask.astype(np.float32), (P, DIM))
    )
    in_maps = []
    for c in range(N_CORES):
        xs = np.ascontiguousarray(x[c * RPC:(c + 1) * RPC], dtype=np.float32)
        cols = []
        for r in range(RPC):
            s = 0
            for t_i in plans[r]:
                cols.append(
                    dense[c * RPC + r, s:s + P * t_i].reshape(P, t_i)
                )
                s += P * t_i
        ms = np.ascontiguousarray(np.concatenate(cols, axis=1))
        assert ms.shape == (P, MASK_COLS)
        in_maps.append({"x": xs, "mask": ms, "emb": emb_b})
    return in_maps


def kernel(x, mask_indices, emb_mask):
    from concourse.bass_utils import run_bass_kernel_spmd

    nc = _get_nc()
    in_maps = _host_prep(
        np.asarray(x), np.asarray(mask_indices), np.asarray(emb_mask)
    )
    res = run_bass_kernel_spmd(nc, in_maps, core_ids=list(range(N_CORES)))
    masked = np.concatenate([r["out"] for r in res.results], axis=0)
    return masked, np.asarray(mask_indices)


# revision 24
# speedup vs baseline: 1.0219x; 1.0219x over previous
"""Masked-fill kernel for Trainium2 (8 NeuronCores, data-parallel over batch).

reference semantics:
    masked_x = x.copy(); masked_x[b, mask_indices[b], :] = emb_mask[0]
    return masked_x, mask_indices

Strategy: shard x along batch (4 rows per core). On the host, expand
mask_indices into a dense {0,1} uint32 mask over the N axis (cheap: 256K
entries; idempotent under duplicate indices). On-device, each core streams
its 32 MiB shard through SBUF in 1 MiB tiles (loads on the sync HWDGE
ring, stores on the scalar HWDGE ring, 20 tile buffers) and applies one
DVE copy_predicated per tile (mask broadcast over DIM, emb broadcast over
the row-chunk axis) between load and store. Total HBM traffic per core =
32 MiB read + 32 MiB write + ~0.25 MiB of mask/emb — the memory roofline
for this op. Measured ~200 us/exec on silicon (~390 GB/s/core combined
read+write), vs ~187 us theoretical at the 358 GB/s HBM-per-core limit.
"""

import numpy as np

B, N, DIM = 32, 8192, 256
N_CORES = 8
RPC = B // N_CORES  # batch rows per core = 4
P = 128             # SBUF partitions
CHUNKS = 8          # tiles per batch row
NH = N // CHUNKS    # 4096 rows of x per tile
T = NH // P         # 32 rows per partition per tile
MASK_COLS = RPC * CHUNKS * T  # 256

_cached = {}


def _chunk_plan(chunks, taper):
    """Per-row list of chunk sizes in t-units (t = rows-per-partition).
    Row plan sums to N//P (=64). With taper, the first row starts with
    small chunks (fast pipeline fill) and the last row ends with small
    chunks (fast drain); middle rows stay uniform."""
    t = N // P // chunks
    base = [t] * chunks
    if not taper:
        return [list(base) for _ in range(RPC)]
    assert chunks == 8, "taper plan is tuned for chunks=8"
    head = [2, 2, 4] + [8] * 7          # sums to 64
    tail = [8] * 7 + [4, 2, 2]          # sums to 64
    plans = [list(base) for _ in range(RPC)]
    plans[0] = head
    plans[RPC - 1] = tail
    return plans


def _build_nc(chunks=CHUNKS, bufs=20, load_eng="sync", store_eng="scalar",
              predicate=True, bench_loop=0, const_eng="scalar", alternate=False,
              third="none", order="rh", taper=False, unroll=1):
    """Build the bass module.

    bench_loop=0: the real kernel (x -> out, one pass).
    bench_loop=K: benchmark variant — a hardware For_i loop runs the body K
    times (x -> scratch), then one small DMA keeps scratch live by writing
    the tiny external output. Used only for timing.
    """
    import concourse.bacc as bacc
    import concourse.mybir as mybir
    import concourse.tile as tile

    nh = N // chunks
    t = nh // P

    nc = bacc.Bacc("TRN2", target_bir_lowering=False, debug=False)
    f32 = mybir.dt.float32
    u32 = mybir.dt.uint32

    x_in = nc.dram_tensor("x", [RPC, N, DIM], f32, kind="ExternalInput")
    mask_in = nc.dram_tensor("mask", [P, MASK_COLS], u32, kind="ExternalInput")
    emb_in = nc.dram_tensor("emb", [P, DIM], f32, kind="ExternalInput")
    if bench_loop:
        out = nc.dram_tensor("out", [P, DIM], f32, kind="ExternalOutput")
        scratch = nc.dram_tensor("scratch", [RPC, N, DIM], f32)
    else:
        out = nc.dram_tensor("out", [RPC, N, DIM], f32, kind="ExternalOutput")
        scratch = None

    with tile.TileContext(nc) as tc:
        with (
            tc.tile_pool(name="const", bufs=1) as cpool,
            tc.tile_pool(name="big", bufs=bufs) as pool,
        ):
            mask_t = cpool.tile([P, MASK_COLS], u32)
            emb_t = cpool.tile([P, DIM], f32)
            cload = getattr(nc, const_eng)
            cload.dma_start(out=mask_t[:], in_=mask_in[:])
            cload.dma_start(out=emb_t[:], in_=emb_in[:])
            load = getattr(nc, load_eng)
            store = getattr(nc, store_eng)

            plans = _chunk_plan(chunks, taper)
            # work item: (row, x-row start, chunk size t_i, mask col start)
            items = []
            for r in range(RPC):
                cum = 0
                for t_i in plans[r]:
                    items.append((r, P * cum, t_i, r * (N // P) + cum))
                    cum += t_i
            if order == "hr":
                assert not taper
                items = [
                    items[r * chunks + h]
                    for h in range(chunks)
                    for r in range(RPC)
                ]

            def body(tgt):
                for i, (r, s, t_i, c0) in enumerate(items):
                    ld = (getattr(nc, store_eng) if (alternate and i % 2)
                          else load)
                    st = (getattr(nc, load_eng) if (alternate and i % 2)
                          else store)
                    if third == "load" and i % 3 == 2:
                        ld = nc.gpsimd
                    elif third == "store" and i % 3 == 2:
                        st = nc.gpsimd
                    nrow = P * t_i
                    buf = pool.tile([P, t * DIM], f32, tag="xbuf")
                    dst3 = buf[:, :t_i * DIM].rearrange(
                        "p (t d) -> p t d", d=DIM
                    )
                    src = x_in[r, s:s + nrow, :].rearrange(
                        "(p t) d -> p t d", p=P
                    )
                    ld.dma_start(out=dst3, in_=src)

                    # mask for x rows [s, s+P*t_i): dense mask for batch
                    # row r occupies cols [r*(N//P), (r+1)*(N//P)); this
                    # chunk starts at col offset cum (= s // P)
                    if predicate:
                        nc.vector.copy_predicated(
                            out=dst3,
                            mask=mask_t[:, c0:c0 + t_i].unsqueeze(2)
                            .broadcast_to([P, t_i, DIM]),
                            data=emb_t[:].unsqueeze(1).broadcast_to(
                                [P, t_i, DIM]
                            ),
                        )

                    dsto = tgt[r, s:s + nrow, :].rearrange(
                        "(p t) d -> p t d", p=P
                    )
                    st.dma_start(out=dsto, in_=dst3)

            if bench_loop:
                with tc.For_i(0, bench_loop, 1):
                    for _ in range(unroll):
                        body(scratch)
                # keep scratch live: copy one row-chunk out through SBUF
                sm = cpool.tile([P, DIM], f32)
                nc.sync.dma_start(
                    out=sm[:],
                    in_=scratch[0, 0:P, :].rearrange("(p o) d -> p (o d)", p=P),
                )
                nc.sync.dma_start(out=out[:], in_=sm[:])
            else:
                body(out)
    nc.compile()
    return nc


def _get_nc():
    if "nc" not in _cached:
        _cached["nc"] = _build_nc()
    return _cached["nc"]


def _host_prep(x, mask_indices, emb_mask, chunks=CHUNKS, taper=False):
    """Build per-core input maps. The mask layout must mirror the device
    chunk plan: for each chunk of t_i rows-per-partition starting at x row
    s, partition p covers rows s + p*t_i + tt, so the chunk's mask block
    is dense[r, s:s+P*t_i].reshape(P, t_i), concatenated along columns."""
    plans = _chunk_plan(chunks, taper)
    dense = np.zeros((B, N), dtype=np.uint32)
    dense[np.arange(B)[:, None], mask_indices.astype(np.int64)] = 1
    emb_b = np.ascontiguousarray(
        np.broadcast_to(emb_mask.astype(np.float32), (P, DIM))
    )
    in_maps = []
    for c in range(N_CORES):
        xs = np.ascontiguousarray(x[c * RPC:(c + 1) * RPC], dtype=np.float32)
        cols = []
        for r in range(RPC):
            s = 0
            for t_i in plans[r]:
                cols.append(
                    dense[c * RPC + r, s:s + P * t_i].reshape(P, t_i)
                )
                s += P * t_i
        ms = np.ascontiguousarray(np.concatenate(cols, axis=1))
        assert ms.shape == (P, MASK_COLS)
        in_maps.append({"x": xs, "mask": ms, "emb": emb_b})
    return in_maps


def kernel(x, mask_indices, emb_mask):
    from concourse.bass_utils import run_bass_kernel_spmd

    nc = _get_nc()
    in_maps = _host_prep(
        np.asarray(x), np.asarray(mask_indices), np.asarray(emb_mask)
    )
    res = run_bass_kernel_spmd(nc, in_maps, core_ids=list(range(N_CORES)))
    masked = np.concatenate([r["out"] for r in res.results], axis=0)
    return masked, np.asarray(mask_indices)
